# revision 11
# baseline (speedup 1.0000x reference)
"""AgentAttention Trainium2 kernel (B=64, N=1024, C=512, M=16 agents) on 8 NeuronCores.

Data-parallel over batch: each core processes 8 batch elements. No collectives.

Math (per batch element, reference semantics):
    Q = x@Wq.T+bq ; K = x@Wk.T+bk ; V = x@Wv.T+bv
    A = group-mean of Q over 64-token groups          -> [16, C]
    S1 = softmax(Q A^T / sqrt(C), axis=agents)        -> [N, 16]
    S2 = softmax(A K^T / sqrt(C), axis=tokens)        -> [16, N]
    out = (S1 @ (S2 @ V)) @ Wo.T + bo

Algebraic restructuring (exact in real arithmetic):
    - bv never materialized: softmax rows sum to 1 =>  out += (Wo@bv + bo) == b'
    - A uses group-SUM; the 1/64 is folded into the logit scale s = 1/(64*sqrt(C))
    - re-association: S1@((S2@x)@(Wv^T Wo^T)) replaces the O(N*C^2) V- and
      output-projections with agent-space (M=16) ops; Wvo^T = (Wo@Wv)^T is
      precomputed on host in float64.
    - the A@bk logit shift is constant along the stage-2 softmax axis and
      cancels; it is dropped entirely.
    - stage-1 softmax normalizer r1 applied as a per-row scale on the final
      output tile; stage-2 normalizer r2 applied when evicting (S2@x) from PSUM.

Perf structure (v2):
    - agent products (Asum, AWQ^T, AWK^T, c1) computed ONCE for all 8 local
      batches with the 8*16=128 (batch,agent) pairs as a full 128-wide matmul
      free/partition dim -- 48 full-width matmuls instead of 448 16-wide ones.
    - AWQ^T/AWK^T stored interleaved as one [128, 32] stationary per
      (c-chunk, batch): stage-1 and stage-2 logits come out of ONE x^T stream.
    - per batch ONE pair of XBAR transposes moves [32, 1024] E1/E2 rows into
      token-partition layout (feeds both the ex matmul and the r1 reduce).
    - all HBM<->SBUF transfers are host-permuted so every SBUF partition
      reads/writes a single contiguous 8KB block (large DMA packets).
"""

import sys

if "/opt/trn_rl_repo" not in sys.path:
    sys.path.insert(0, "/opt/trn_rl_repo")

import numpy as np
import ml_dtypes

import concourse.bass as bass
import concourse.mybir as mybir
import concourse.tile as tile
from concourse import bacc
from concourse.bass import ts, ds
from concourse.bass_utils import run_bass_kernel_spmd
from concourse.masks import make_identity

BF16 = mybir.dt.bfloat16
F32 = mybir.dt.float32
F8 = mybir.dt.float8e4

N_CORES = 8
B = 64
B_LOC = B // N_CORES  # 8 batches per core
N = 1024              # tokens
C = 512               # channels
M = 16                # agents
G = N // M            # 64-token pooling groups
P = 128
ND = C // P           # 4 channel chunks
NN = N // P           # 8 token chunks of 128
J = B_LOC * M         # 128 stacked (batch, agent) columns
SCALE = 1.0 / (G * np.sqrt(C))  # logit scale (1/64 pooling fold included)

# test harness may override (e.g. {"trace": True, "tmpdir": ...})
_RUN_KWARGS = {}
_LAST_RESULTS = None


def _build_program():
    nc = bacc.Bacc("TRN2", target_bir_lowering=False, debug=False,
                   num_devices=N_CORES)

    xt_d = nc.dram_tensor("xt", [B_LOC, P, 2, 2, N], F8, kind="ExternalInput")
    xn_d = nc.dram_tensor("xn", [B_LOC, P, NN, C], BF16, kind="ExternalInput")
    xs_d = nc.dram_tensor("xs", [P, ND, J], BF16, kind="ExternalInput")
    wqT_d = nc.dram_tensor("wqT", [P, ND, C], BF16, kind="ExternalInput")
    wqN_d = nc.dram_tensor("wqN", [P, ND, C], BF16, kind="ExternalInput")
    wkN_d = nc.dram_tensor("wkN", [P, ND, C], BF16, kind="ExternalInput")
    wvo_d = nc.dram_tensor("wvoT", [P, ND, C], BF16, kind="ExternalInput")
    bq64_d = nc.dram_tensor("bq64", [P, ND], F32, kind="ExternalInput")
    bp_d = nc.dram_tensor("bp", [P, C], BF16, kind="ExternalInput")
    c12b_d = nc.dram_tensor("c12b", [3 * M, B_LOC], F32, kind="ExternalInput")
    out_d = nc.dram_tensor("out", [B_LOC, P, NN, C], BF16, kind="ExternalOutput")

    with tile.TileContext(nc) as tc:
        with (
            tc.tile_pool(name="const", bufs=1) as const,
            tc.tile_pool(name="pxt", bufs=4) as pxt,
            tc.tile_pool(name="pxn", bufs=4) as pxn,
            tc.tile_pool(name="pe12", bufs=3) as pe12,
            tc.tile_pool(name="pe12p", bufs=3) as pe12p,
            tc.tile_pool(name="psmall", bufs=4) as psmall,
            tc.tile_pool(name="pout", bufs=3) as pout,
            tc.tile_pool(name="ps_big", bufs=4, space="PSUM") as ps_big,
            tc.tile_pool(name="ps_log", bufs=2, space="PSUM") as ps_log,
            tc.tile_pool(name="ps_se", bufs=2, space="PSUM") as ps_se,
        ):
            wqT_s = const.tile([P, ND, C], BF16)
            wqN_s = const.tile([P, ND, C], BF16)
            wkN_s = const.tile([P, ND, C], BF16)
            wvo_s = const.tile([P, ND, C], BF16)
            xs_s = const.tile([P, ND, J], BF16)
            bq64_s = const.tile([P, ND], F32)
            bp_s = const.tile([P, C], BF16)
            asum_s = const.tile([P, ND, J], BF16)
            awqk_s = const.tile([P, 2, 2, B_LOC, 3 * M], F8)
            c12b_s = const.tile([3 * M, B_LOC], F32)
            ident = const.tile([M, M], BF16)

            # first wave of const loads: everything the agent stage needs
            nc.sync.dma_start(wqT_s[:], wqT_d.ap())
            nc.sync.dma_start(xs_s[:], xs_d.ap())
            nc.sync.dma_start(bq64_s[:], bq64_d.ap())
            make_identity(nc, ident)
            nc.vector.memset(awqk_s[:, :, :, :, M:2 * M], 0.0)

            st = [dict() for _ in range(B_LOC)]

            def load_x(b):
                s = st[b]
                s["xt"] = xt = pxt.tile([P, 2, 2, N], F8, tag="xt", name=f"xt_{b}")
                nc.sync.dma_start(xt[:], xt_d.ap()[b])
                s["xn"] = xn = pxn.tile([P, NN, C], BF16, tag="xn", name=f"xn_{b}")
                nc.sync.dma_start(xn[:], xn_d.ap()[b])

            def agent_stage():
                # AsumT[d, j] = sum_c Wq^T[c, d] xsum^T[c, j] + 64 bq[d]
                for d in range(ND):
                    ps = ps_log.tile([P, B_LOC, M], F32, tag="log", name="ag")
                    for c in range(ND):
                        nc.tensor.matmul(
                            ps[:], wqT_s[:, c, ds(d * P, P)], xs_s[:, c, :],
                            start=(c == 0), stop=(c == ND - 1))
                    nc.scalar.activation(
                        asum_s[:, d, :], ps[:],
                        mybir.ActivationFunctionType.Identity,
                        bias=bq64_s[:, d:d + 1])
                # AWQ^T[c, j] and AWK^T[c, j], interleaved per batch as the
                # fused [128, 32] logit stationary
                for (w_s, half) in ((wqN_s, 0), (wkN_s, 2)):
                    for c in range(ND):
                        ps = ps_log.tile([P, B_LOC, M], F32, tag="log", name="ag")
                        for d in range(ND):
                            nc.tensor.matmul(
                                ps[:], w_s[:, d, ds(c * P, P)], asum_s[:, d, :],
                                start=(d == 0), stop=(d == ND - 1))
                        nc.scalar.activation(
                            awqk_s[:, c // 2, c % 2, :,
                                   half * M:(half + 1) * M], ps[:],
                            mybir.ActivationFunctionType.Copy)

            def l12(b):
                # E12^T[0:16, n] = exp(s*Q A^T + c1) ; [16:32, n] = exp(s*A K^T)
                s = st[b]
                s["e12t"] = e12t = pe12.tile([3 * M, N], BF16, tag="e12t",
                                             name=f"e12t_{b}")
                d2 = psmall.tile([3 * M, 2], F32, tag="d2", name=f"d2_{b}")
                for ni in range(2):
                    lg = ps_log.tile([3 * M, 512], F32, tag="log", name=f"log_{b}")
                    for kk in range(2):
                        nc.tensor.matmul(
                            lg[:], awqk_s[:, kk, :, b, :],
                            s["xt"][:, kk, :, ts(ni, 512)],
                            start=(kk == 0), stop=(kk == 1),
                            perf_mode=mybir.MatmulPerfMode.DoubleRow)
                    nc.scalar.activation(
                        e12t[:, ts(ni, 512)], lg[:],
                        mybir.ActivationFunctionType.Exp,
                        bias=c12b_s[:, b:b + 1], scale=float(SCALE),
                        accum_out=d2[:, ni:ni + 1])
                s["d2"] = d2

            def r2c(b):
                s = st[b]
                d2 = s["d2"]
                d2s = psmall.tile([M, 1], F32, tag="d2s", name=f"d2s_{b}")
                nc.vector.tensor_add(d2s[:], d2[2 * M:3 * M, 0:1], d2[2 * M:3 * M, 1:2])
                s["r2"] = r2 = psmall.tile([M, 1], F32, tag="r2", name=f"r2_{b}")
                nc.vector.reciprocal(r2[:], d2s[:])

            def tr(b):
                # e12p[p, o, i] = E12^T[i, o*128+p] via XBAR dma transpose
                s = st[b]
                s["e12p"] = e12p = pe12p.tile([P, NN, 3 * M], BF16, tag="e12p",
                                              name=f"e12p_{b}")
                e12t = s["e12t"]
                h = NN // 2
                nc.sync.dma_start_transpose(e12p[:, 0:h, :], e12t[:, 0:512])
                nc.sync.dma_start_transpose(e12p[:, h:NN, :], e12t[:, 512:])

            def r1(b):
                # r1[n] = sum_m E1^T[m, n]; free-dim reduce in token-partition
                s = st[b]
                r_s = psmall.tile([P, NN], F32, tag="r_s", name=f"r_s_{b}")
                nc.vector.reduce_sum(r_s[:], s["e12p"][:, :, 0:M],
                                     axis=mybir.AxisListType.X)
                s["r_inv"] = r_inv = psmall.tile([P, NN], F32, tag="r_inv",
                                                 name=f"r_inv_{b}")
                nc.vector.reciprocal(r_inv[:], r_s[:])

            def ex(b):
                # exr = diag(r2) * (E2 @ x)   [16, C]
                s = st[b]
                pse = ps_se.tile([M, C], F32, tag="se", name=f"se_{b}")
                for n in range(NN):
                    nc.tensor.matmul(
                        pse[:], s["e12p"][:, n, 2 * M:3 * M], s["xn"][:, n, :],
                        start=(n == 0), stop=(n == NN - 1))
                s["exr"] = exr = psmall.tile([M, C], BF16, tag="exr",
                                             name=f"exr_{b}")
                nc.scalar.activation(
                    exr[:], pse[:], mybir.ActivationFunctionType.Copy,
                    scale=s["r2"][:])

            def exrT(b):
                # exrT[p, c, m] = exr[m, c*128+p] via PE transposes
                s = st[b]
                pst = ps_big.tile([P, ND, M], BF16, tag="mm", name=f"tr_{b}")
                for c in range(ND):
                    nc.tensor.transpose(pst[:, c, :], s["exr"][:, ts(c, P)],
                                        ident[:])
                s["exrT"] = exrT_ = psmall.tile([P, ND, M], BF16, tag="exrT",
                                                name=f"exrT_{b}")
                nc.vector.tensor_copy(exrT_[:], pst[:])

            def afw(b):
                # afw = exr @ Wvo^T   [16, C]
                s = st[b]
                psa = ps_se.tile([M, C], F32, tag="se", name=f"afw_{b}")
                for c in range(ND):
                    nc.tensor.matmul(
                        psa[:], s["exrT"][:, c, :], wvo_s[:, c, :],
                        start=(c == 0), stop=(c == ND - 1))
                s["afw"] = afw_ = psmall.tile([M, C], BF16, tag="afw",
                                              name=f"afw_{b}")
                nc.scalar.activation(
                    afw_[:], psa[:], mybir.ActivationFunctionType.Copy)
                nc.vector.tensor_add(afw_[:], afw_[:], bp_s[0:M, :])

            def outp(b, rng=None):
                # out chunk = r1 * (E1 @ afw) + b'
                s = st[b]
                if rng is None:
                    s["o"] = o_s = pout.tile([P, NN, C], BF16, tag="o", name=f"o_{b}")
                    rng = range(NN)
                else:
                    if "o" not in s:
                        s["o"] = pout.tile([P, NN, C], BF16, tag="o", name=f"o_{b}")
                    o_s = s["o"]
                for n in rng:
                    po = ps_big.tile([P, C], F32, tag="mm", name=f"mm_{b}")
                    nc.tensor.matmul(
                        po[:], s["e12t"][0:M, ts(n, P)], s["afw"][:],
                        start=True, stop=True)
                    if n % 2 == 0:
                        nc.scalar.activation(
                            o_s[:, n, :], po[:],
                            mybir.ActivationFunctionType.Copy,
                            scale=s["r_inv"][:, n:n + 1])
                    else:
                        nc.vector.tensor_scalar_mul(
                            o_s[:, n, :], po[:], s["r_inv"][:, n:n + 1])

            def store(b, half=None):
                if half is None:
                    nc.sync.dma_start(out_d.ap()[b], st[b]["o"][:])
                else:
                    h = NN // 2
                    sl = slice(half * h, (half + 1) * h)
                    nc.sync.dma_start(out_d.ap()[b][:, sl], st[b]["o"][:, sl])

            # prologue: agent products overlap the first x loads
            load_x(0)
            nc.sync.dma_start(wqN_s[:], wqN_d.ap())
            nc.sync.dma_start(wkN_s[:], wkN_d.ap())
            nc.sync.dma_start(c12b_s[:], c12b_d.ap())
            agent_stage()
            load_x(1)
            nc.sync.dma_start(wvo_s[:], wvo_d.ap())
            nc.sync.dma_start(bp_s[:], bp_d.ap())
            load_x(2)

            for b in range(B_LOC):
                l12(b)
                tr(b)
                if b + 3 < B_LOC:
                    load_x(b + 3)
                if b >= 1:
                    ex(b - 1)
                if b >= 2:
                    outp(b - 2, range(0, NN // 2))
                    store(b - 2, 0)
                    outp(b - 2, range(NN // 2, NN))
                    store(b - 2, 1)
                if b >= 1:
                    exrT(b - 1)
                    afw(b - 1)
                r2c(b)
                r1(b)
            L = B_LOC - 1
            ex(L)
            outp(L - 1, range(0, NN // 2))
            store(L - 1, 0)
            outp(L - 1, range(NN // 2, NN))
            store(L - 1, 1)
            exrT(L)
            afw(L)
            outp(L, range(0, NN // 2))
            store(L, 0)
            outp(L, range(NN // 2, NN))
            store(L, 1)

    nc.compile()
    return nc


def _prep_inputs(x, Wq, bq, Wk, bk, Wv, bv, Wo, bo):
    bf = ml_dtypes.bfloat16
    x32 = np.asarray(x, np.float32)
    # xt[b, p, kk, i, n] = x[b, n, kk*256+i*128+p]  (fp8 DoubleRow planes)
    f8 = ml_dtypes.float8_e4m3
    xt = np.ascontiguousarray(
        x32.transpose(0, 2, 1).reshape(B, 2, 2, P, N).transpose(0, 3, 1, 2, 4)
    ).astype(f8)
    # xn[b, p, o, c] = x[b, o*128+p, c]
    xn = np.ascontiguousarray(
        x32.reshape(B, NN, P, C).transpose(0, 2, 1, 3)).astype(bf)
    # pooled sums, all local batches stacked: xs[p, o, b*16+m]
    xsum = x32.reshape(B, M, G, C).sum(axis=2)  # [B, M, C]
    Wo64 = np.asarray(Wo, np.float64)
    Wv64 = np.asarray(Wv, np.float64)

    def wtile(w):  # [C, C] -> [P, ND, C] with w[p, o, d] = W[o*128+p, d]
        return np.ascontiguousarray(
            np.asarray(w, np.float32).reshape(ND, P, C).transpose(1, 0, 2)
        ).astype(bf)

    shared = {
        "wqT": wtile(np.asarray(Wq, np.float32).T),
        "wqN": wtile(np.asarray(Wq, np.float32)),
        "wkN": wtile(np.asarray(Wk, np.float32)),
        "wvoT": wtile((Wo64 @ Wv64).T.astype(np.float32)),
        "bq64": np.ascontiguousarray(
            (64.0 * np.asarray(bq, np.float32)).reshape(ND, P).T),
    }
    bprime = np.asarray(bo, np.float64) + Wo64 @ np.asarray(bv, np.float64)
    shared["bp"] = np.tile(bprime.astype(np.float32), (P, 1)).astype(bf)
    in_maps = []
    for ci in range(N_CORES):
        m = dict(shared)
        m["xt"] = np.ascontiguousarray(xt[ci * B_LOC:(ci + 1) * B_LOC])
        m["xn"] = np.ascontiguousarray(xn[ci * B_LOC:(ci + 1) * B_LOC])
        xs_c = xsum[ci * B_LOC:(ci + 1) * B_LOC]  # [B_LOC, M, C]
        xs_j = xs_c.transpose(2, 0, 1).reshape(C, J)  # [C, J], j = b*16+m
        m["xs"] = np.ascontiguousarray(
            xs_j.reshape(ND, P, J).transpose(1, 0, 2)).astype(bf)
        # stage-1 logit bias c1[b, m] = SCALE * (Asum_b,m . bq), exact on host
        asum64 = xs_c.astype(np.float64) @ np.asarray(Wq, np.float64).T \
            + 64.0 * np.asarray(bq, np.float64)
        c1 = SCALE * (asum64 @ np.asarray(bq, np.float64))  # [B_LOC, M]
        c12b = np.zeros((3 * M, B_LOC), np.float32)
        c12b[0:M, :] = c1.T.astype(np.float32)
        m["c12b"] = c12b
        in_maps.append(m)
    return in_maps


def _unpermute_out(res):
    # out_d[b, p, o, c] = out[b, o*128+p, c]
    outs = []
    for ci in range(N_CORES):
        o = np.asarray(res.results[ci]["out"], np.float32)  # [B_LOC, P, NN, C]
        outs.append(o.transpose(0, 2, 1, 3).reshape(B_LOC, N, C))
    return np.concatenate(outs, axis=0)


def kernel(x, Wq, bq, Wk, bk, Wv, bv, Wo, bo):
    global _LAST_RESULTS
    nc = _build_program()
    in_maps = _prep_inputs(x, Wq, bq, Wk, bk, Wv, bv, Wo, bo)
    res = run_bass_kernel_spmd(nc, in_maps, list(range(N_CORES)), **_RUN_KWARGS)
    _LAST_RESULTS = res
    return _unpermute_out(res)


# revision 12
# speedup vs baseline: 1.0689x; 1.0689x over previous
"""AgentAttention Trainium2 kernel (B=64, N=1024, C=512, M=16 agents) on 8 NeuronCores.

Data-parallel over batch: each core processes 8 batch elements. No collectives.

Math (per batch element, reference semantics):
    Q = x@Wq.T+bq ; K = x@Wk.T+bk ; V = x@Wv.T+bv
    A = group-mean of Q over 64-token groups          -> [16, C]
    S1 = softmax(Q A^T / sqrt(C), axis=agents)        -> [N, 16]
    S2 = softmax(A K^T / sqrt(C), axis=tokens)        -> [16, N]
    out = (S1 @ (S2 @ V)) @ Wo.T + bo

Algebraic restructuring (exact in real arithmetic):
    - bv never materialized: softmax rows sum to 1 =>  out += (Wo@bv + bo) == b'
    - A uses group-SUM; the 1/64 is folded into the logit scale s = 1/(64*sqrt(C))
    - re-association: S1@((S2@x)@(Wv^T Wo^T)) replaces the O(N*C^2) V- and
      output-projections with agent-space (M=16) ops; Wvo^T = (Wo@Wv)^T is
      precomputed on host in float64.
    - the A@bk logit shift is constant along the stage-2 softmax axis and
      cancels; it is dropped entirely.
    - stage-1 softmax normalizer r1 applied as a per-row scale on the final
      output tile; stage-2 normalizer r2 applied when evicting (S2@x) from PSUM.

Perf structure (v2):
    - agent products (Asum, AWQ^T, AWK^T, c1) computed ONCE for all 8 local
      batches with the 8*16=128 (batch,agent) pairs as a full 128-wide matmul
      free/partition dim -- 48 full-width matmuls instead of 448 16-wide ones.
    - AWQ^T/AWK^T stored interleaved as one [128, 32] stationary per
      (c-chunk, batch): stage-1 and stage-2 logits come out of ONE x^T stream.
    - per batch ONE pair of XBAR transposes moves [32, 1024] E1/E2 rows into
      token-partition layout (feeds both the ex matmul and the r1 reduce).
    - all HBM<->SBUF transfers are host-permuted so every SBUF partition
      reads/writes a single contiguous 8KB block (large DMA packets).
"""

import sys

if "/opt/trn_rl_repo" not in sys.path:
    sys.path.insert(0, "/opt/trn_rl_repo")

import numpy as np
import ml_dtypes

import concourse.bass as bass
import concourse.mybir as mybir
import concourse.tile as tile
from concourse import bacc
from concourse.bass import ts, ds
from concourse.bass_utils import run_bass_kernel_spmd
from concourse.masks import make_identity

BF16 = mybir.dt.bfloat16
F32 = mybir.dt.float32
F8 = mybir.dt.float8e4

N_CORES = 8
B = 64
B_LOC = B // N_CORES  # 8 batches per core
N = 1024              # tokens
C = 512               # channels
M = 16                # agents
G = N // M            # 64-token pooling groups
P = 128
ND = C // P           # 4 channel chunks
NN = N // P           # 8 token chunks of 128
J = B_LOC * M         # 128 stacked (batch, agent) columns
SCALE = 1.0 / (G * np.sqrt(C))  # logit scale (1/64 pooling fold included)

# test harness may override (e.g. {"trace": True, "tmpdir": ...})
_RUN_KWARGS = {}
_LAST_RESULTS = None


def _build_program():
    nc = bacc.Bacc("TRN2", target_bir_lowering=False, debug=False,
                   num_devices=N_CORES)

    xt_d = nc.dram_tensor("xt", [B_LOC, P, 2, 2, N], F8, kind="ExternalInput")
    xn_d = nc.dram_tensor("xn", [B_LOC, P, NN, C], BF16, kind="ExternalInput")
    xs_d = nc.dram_tensor("xs", [P, ND, J], BF16, kind="ExternalInput")
    wqT_d = nc.dram_tensor("wqT", [P, ND, C], BF16, kind="ExternalInput")
    wqN_d = nc.dram_tensor("wqN", [P, ND, C], BF16, kind="ExternalInput")
    wkN_d = nc.dram_tensor("wkN", [P, ND, C], BF16, kind="ExternalInput")
    wvo_d = nc.dram_tensor("wvoT", [P, ND, C], BF16, kind="ExternalInput")
    bq64_d = nc.dram_tensor("bq64", [P, ND], F32, kind="ExternalInput")
    bp_d = nc.dram_tensor("bp", [P, C], BF16, kind="ExternalInput")
    c12b_d = nc.dram_tensor("c12b", [3 * M, B_LOC], F32, kind="ExternalInput")
    out_d = nc.dram_tensor("out", [B_LOC, P, NN, C], BF16, kind="ExternalOutput")

    with tile.TileContext(nc) as tc:
        with (
            tc.tile_pool(name="const", bufs=1) as const,
            tc.tile_pool(name="pxt", bufs=4) as pxt,
            tc.tile_pool(name="pxn", bufs=4) as pxn,
            tc.tile_pool(name="pe12", bufs=3) as pe12,
            tc.tile_pool(name="pe12p", bufs=3) as pe12p,
            tc.tile_pool(name="psmall", bufs=4) as psmall,
            tc.tile_pool(name="pout", bufs=3) as pout,
            tc.tile_pool(name="ps_big", bufs=4, space="PSUM") as ps_big,
            tc.tile_pool(name="ps_log", bufs=2, space="PSUM") as ps_log,
            tc.tile_pool(name="ps_se", bufs=2, space="PSUM") as ps_se,
        ):
            wqT_s = const.tile([P, ND, C], BF16)
            wqN_s = const.tile([P, ND, C], BF16)
            wkN_s = const.tile([P, ND, C], BF16)
            wvo_s = const.tile([P, ND, C], BF16)
            xs_s = const.tile([P, ND, J], BF16)
            bq64_s = const.tile([P, ND], F32)
            bp_s = const.tile([P, C], BF16)
            asum_s = const.tile([P, ND, J], BF16)
            awqk_s = const.tile([P, 2, 2, B_LOC, 3 * M], F8)
            c12b_s = const.tile([3 * M, B_LOC], F32)
            ident = const.tile([M, M], BF16)

            # first wave of const loads: everything the agent stage needs
            nc.sync.dma_start(wqT_s[:], wqT_d.ap())
            nc.sync.dma_start(xs_s[:], xs_d.ap())
            nc.sync.dma_start(bq64_s[:], bq64_d.ap())
            make_identity(nc, ident)
            nc.vector.memset(awqk_s[:, :, :, :, M:2 * M], 0.0)

            st = [dict() for _ in range(B_LOC)]

            def load_x(b):
                s = st[b]
                s["xt"] = xt = pxt.tile([P, 2, 2, N], F8, tag="xt", name=f"xt_{b}")
                nc.sync.dma_start(xt[:], xt_d.ap()[b])
                s["xn"] = xn = pxn.tile([P, NN, C], BF16, tag="xn", name=f"xn_{b}")
                nc.sync.dma_start(xn[:], xn_d.ap()[b])

            def agent_stage():
                # AsumT[d, j] = sum_c Wq^T[c, d] xsum^T[c, j] + 64 bq[d]
                for d in range(ND):
                    ps = ps_big.tile([P, B_LOC, M], F32, tag="mm", name="ag")
                    for c in range(ND):
                        nc.tensor.matmul(
                            ps[:], wqT_s[:, c, ds(d * P, P)], xs_s[:, c, :],
                            start=(c == 0), stop=(c == ND - 1))
                    nc.scalar.activation(
                        asum_s[:, d, :], ps[:],
                        mybir.ActivationFunctionType.Identity,
                        bias=bq64_s[:, d:d + 1])
                # AWQ^T[c, j] and AWK^T[c, j], interleaved per batch as the
                # fused [128, 32] logit stationary
                for (w_s, half) in ((wqN_s, 0), (wkN_s, 2)):
                    for c in range(ND):
                        ps = ps_big.tile([P, B_LOC, M], F32, tag="mm", name="ag")
                        for d in range(ND):
                            nc.tensor.matmul(
                                ps[:], w_s[:, d, ds(c * P, P)], asum_s[:, d, :],
                                start=(d == 0), stop=(d == ND - 1))
                        nc.scalar.activation(
                            awqk_s[:, c // 2, c % 2, :,
                                   half * M:(half + 1) * M], ps[:],
                            mybir.ActivationFunctionType.Copy)

            def l12(b):
                # E12^T[0:16, n] = exp(s*Q A^T + c1) ; [16:32, n] = exp(s*A K^T)
                s = st[b]
                s["e12t"] = e12t = pe12.tile([3 * M, N], BF16, tag="e12t",
                                             name=f"e12t_{b}")
                d2 = psmall.tile([3 * M, 2], F32, tag="d2", name=f"d2_{b}")
                for ni in range(2):
                    lg = ps_log.tile([3 * M, 512], F32, tag="log", name=f"log_{b}")
                    for kk in range(2):
                        nc.tensor.matmul(
                            lg[:], awqk_s[:, kk, :, b, :],
                            s["xt"][:, kk, :, ts(ni, 512)],
                            start=(kk == 0), stop=(kk == 1),
                            perf_mode=mybir.MatmulPerfMode.DoubleRow)
                    nc.scalar.activation(
                        e12t[:, ts(ni, 512)], lg[:],
                        mybir.ActivationFunctionType.Exp,
                        bias=c12b_s[:, b:b + 1], scale=float(SCALE),
                        accum_out=d2[:, ni:ni + 1])
                s["d2"] = d2

            def r2c(b):
                s = st[b]
                d2 = s["d2"]
                d2s = psmall.tile([M, 1], F32, tag="d2s", name=f"d2s_{b}")
                nc.vector.tensor_add(d2s[:], d2[2 * M:3 * M, 0:1], d2[2 * M:3 * M, 1:2])
                s["r2"] = r2 = psmall.tile([M, 1], F32, tag="r2", name=f"r2_{b}")
                nc.vector.reciprocal(r2[:], d2s[:])

            def tr(b):
                # e12p[p, o, i] = E12^T[i, o*128+p] via XBAR dma transpose
                s = st[b]
                s["e12p"] = e12p = pe12p.tile([P, NN, 3 * M], BF16, tag="e12p",
                                              name=f"e12p_{b}")
                e12t = s["e12t"]
                h = NN // 2
                nc.sync.dma_start_transpose(e12p[:, 0:h, :], e12t[:, 0:512])
                nc.sync.dma_start_transpose(e12p[:, h:NN, :], e12t[:, 512:])

            def r1(b):
                # r1[n] = sum_m E1^T[m, n]; free-dim reduce in token-partition
                s = st[b]
                r_s = psmall.tile([P, NN], F32, tag="r_s", name=f"r_s_{b}")
                nc.vector.reduce_sum(r_s[:], s["e12p"][:, :, 0:M],
                                     axis=mybir.AxisListType.X)
                s["r_inv"] = r_inv = psmall.tile([P, NN], F32, tag="r_inv",
                                                 name=f"r_inv_{b}")
                nc.vector.reciprocal(r_inv[:], r_s[:])

            def ex(b):
                # exr = diag(r2) * (E2 @ x)   [16, C]
                s = st[b]
                pse = ps_se.tile([M, C], F32, tag="se", name=f"se_{b}")
                for n in range(NN):
                    nc.tensor.matmul(
                        pse[:], s["e12p"][:, n, 2 * M:3 * M], s["xn"][:, n, :],
                        start=(n == 0), stop=(n == NN - 1))
                s["exr"] = exr = psmall.tile([M, C], BF16, tag="exr",
                                             name=f"exr_{b}")
                nc.scalar.activation(
                    exr[:], pse[:], mybir.ActivationFunctionType.Copy,
                    scale=s["r2"][:])

            def exrT(b):
                # exrT[p, c, m] = exr[m, c*128+p] via PE transposes
                s = st[b]
                pst = ps_se.tile([P, ND, M], BF16, tag="se", name=f"tr_{b}")
                for c in range(ND):
                    nc.tensor.transpose(pst[:, c, :], s["exr"][:, ts(c, P)],
                                        ident[:])
                s["exrT"] = exrT_ = psmall.tile([P, ND, M], BF16, tag="exrT",
                                                name=f"exrT_{b}")
                nc.vector.tensor_copy(exrT_[:], pst[:])

            def afw(b):
                # afw = exr @ Wvo^T   [16, C]
                s = st[b]
                psa = ps_se.tile([M, C], F32, tag="se", name=f"afw_{b}")
                for c in range(ND):
                    nc.tensor.matmul(
                        psa[:], s["exrT"][:, c, :], wvo_s[:, c, :],
                        start=(c == 0), stop=(c == ND - 1))
                s["afw"] = afw_ = psmall.tile([M, C], BF16, tag="afw",
                                              name=f"afw_{b}")
                nc.scalar.activation(
                    afw_[:], psa[:], mybir.ActivationFunctionType.Copy)
                nc.vector.tensor_add(afw_[:], afw_[:], bp_s[0:M, :])

            def outp(b, rng=None):
                # out chunk = r1 * (E1 @ afw) + b'
                s = st[b]
                if rng is None:
                    s["o"] = o_s = pout.tile([P, NN, C], BF16, tag="o", name=f"o_{b}")
                    rng = range(NN)
                else:
                    if "o" not in s:
                        s["o"] = pout.tile([P, NN, C], BF16, tag="o", name=f"o_{b}")
                    o_s = s["o"]
                for n in rng:
                    po = ps_big.tile([P, C], F32, tag="mm", name=f"mm_{b}")
                    nc.tensor.matmul(
                        po[:], s["e12t"][0:M, ts(n, P)], s["afw"][:],
                        start=True, stop=True)
                    if n % 2 == 0:
                        nc.scalar.activation(
                            o_s[:, n, :], po[:],
                            mybir.ActivationFunctionType.Copy,
                            scale=s["r_inv"][:, n:n + 1])
                    else:
                        nc.vector.tensor_scalar_mul(
                            o_s[:, n, :], po[:], s["r_inv"][:, n:n + 1])

            def store(b, half=None):
                if half is None:
                    nc.sync.dma_start(out_d.ap()[b], st[b]["o"][:])
                else:
                    h = NN // 2
                    sl = slice(half * h, (half + 1) * h)
                    nc.sync.dma_start(out_d.ap()[b][:, sl], st[b]["o"][:, sl])

            # prologue: agent products overlap the first x loads
            load_x(0)
            nc.sync.dma_start(wqN_s[:], wqN_d.ap())
            nc.sync.dma_start(wkN_s[:], wkN_d.ap())
            nc.sync.dma_start(c12b_s[:], c12b_d.ap())
            agent_stage()
            load_x(1)
            nc.sync.dma_start(wvo_s[:], wvo_d.ap())
            nc.sync.dma_start(bp_s[:], bp_d.ap())
            load_x(2)

            for b in range(B_LOC):
                l12(b)
                tr(b)
                if b + 3 < B_LOC:
                    load_x(b + 3)
                if b >= 1:
                    ex(b - 1)
                if b >= 2:
                    outp(b - 2)
                    store(b - 2)
                if b >= 1:
                    exrT(b - 1)
                    afw(b - 1)
                r2c(b)
                r1(b)
            L = B_LOC - 1
            ex(L)
            outp(L - 1)
            store(L - 1)
            exrT(L)
            afw(L)
            outp(L, range(0, NN // 2))
            store(L, 0)
            outp(L, range(NN // 2, NN))
            store(L, 1)

    nc.compile()
    return nc


def _prep_inputs(x, Wq, bq, Wk, bk, Wv, bv, Wo, bo):
    bf = ml_dtypes.bfloat16
    x32 = np.asarray(x, np.float32)
    # xt[b, p, kk, i, n] = x[b, n, kk*256+i*128+p]  (fp8 DoubleRow planes)
    f8 = ml_dtypes.float8_e4m3
    xt = np.ascontiguousarray(
        x32.transpose(0, 2, 1).reshape(B, 2, 2, P, N).transpose(0, 3, 1, 2, 4)
    ).astype(f8)
    # xn[b, p, o, c] = x[b, o*128+p, c]
    xn = np.ascontiguousarray(
        x32.reshape(B, NN, P, C).transpose(0, 2, 1, 3)).astype(bf)
    # pooled sums, all local batches stacked: xs[p, o, b*16+m]
    xsum = x32.reshape(B, M, G, C).sum(axis=2)  # [B, M, C]
    Wo64 = np.asarray(Wo, np.float64)
    Wv64 = np.asarray(Wv, np.float64)

    def wtile(w):  # [C, C] -> [P, ND, C] with w[p, o, d] = W[o*128+p, d]
        return np.ascontiguousarray(
            np.asarray(w, np.float32).reshape(ND, P, C).transpose(1, 0, 2)
        ).astype(bf)

    shared = {
        "wqT": wtile(np.asarray(Wq, np.float32).T),
        "wqN": wtile(np.asarray(Wq, np.float32)),
        "wkN": wtile(np.asarray(Wk, np.float32)),
        "wvoT": wtile((Wo64 @ Wv64).T.astype(np.float32)),
        "bq64": np.ascontiguousarray(
            (64.0 * np.asarray(bq, np.float32)).reshape(ND, P).T),
    }
    bprime = np.asarray(bo, np.float64) + Wo64 @ np.asarray(bv, np.float64)
    shared["bp"] = np.tile(bprime.astype(np.float32), (P, 1)).astype(bf)
    in_maps = []
    for ci in range(N_CORES):
        m = dict(shared)
        m["xt"] = np.ascontiguousarray(xt[ci * B_LOC:(ci + 1) * B_LOC])
        m["xn"] = np.ascontiguousarray(xn[ci * B_LOC:(ci + 1) * B_LOC])
        xs_c = xsum[ci * B_LOC:(ci + 1) * B_LOC]  # [B_LOC, M, C]
        xs_j = xs_c.transpose(2, 0, 1).reshape(C, J)  # [C, J], j = b*16+m
        m["xs"] = np.ascontiguousarray(
            xs_j.reshape(ND, P, J).transpose(1, 0, 2)).astype(bf)
        # stage-1 logit bias c1[b, m] = SCALE * (Asum_b,m . bq), exact on host
        asum64 = xs_c.astype(np.float64) @ np.asarray(Wq, np.float64).T \
            + 64.0 * np.asarray(bq, np.float64)
        c1 = SCALE * (asum64 @ np.asarray(bq, np.float64))  # [B_LOC, M]
        c12b = np.zeros((3 * M, B_LOC), np.float32)
        c12b[0:M, :] = c1.T.astype(np.float32)
        m["c12b"] = c12b
        in_maps.append(m)
    return in_maps


def _unpermute_out(res):
    # out_d[b, p, o, c] = out[b, o*128+p, c]
    outs = []
    for ci in range(N_CORES):
        o = np.asarray(res.results[ci]["out"], np.float32)  # [B_LOC, P, NN, C]
        outs.append(o.transpose(0, 2, 1, 3).reshape(B_LOC, N, C))
    return np.concatenate(outs, axis=0)


def kernel(x, Wq, bq, Wk, bk, Wv, bv, Wo, bo):
    global _LAST_RESULTS
    nc = _build_program()
    in_maps = _prep_inputs(x, Wq, bq, Wk, bk, Wv, bv, Wo, bo)
    res = run_bass_kernel_spmd(nc, in_maps, list(range(N_CORES)), **_RUN_KWARGS)
    _LAST_RESULTS = res
    return _unpermute_out(res)


# revision 13
# speedup vs baseline: 1.0710x; 1.0019x over previous
"""AgentAttention Trainium2 kernel (B=64, N=1024, C=512, M=16 agents) on 8 NeuronCores.

Data-parallel over batch: each core processes 8 batch elements. No collectives.

Math (per batch element, reference semantics):
    Q = x@Wq.T+bq ; K = x@Wk.T+bk ; V = x@Wv.T+bv
    A = group-mean of Q over 64-token groups          -> [16, C]
    S1 = softmax(Q A^T / sqrt(C), axis=agents)        -> [N, 16]
    S2 = softmax(A K^T / sqrt(C), axis=tokens)        -> [16, N]
    out = (S1 @ (S2 @ V)) @ Wo.T + bo

Algebraic restructuring (exact in real arithmetic):
    - bv never materialized: softmax rows sum to 1 =>  out += (Wo@bv + bo) == b'
    - A uses group-SUM; the 1/64 is folded into the logit scale s = 1/(64*sqrt(C))
    - re-association: S1@((S2@x)@(Wv^T Wo^T)) replaces the O(N*C^2) V- and
      output-projections with agent-space (M=16) ops; Wvo^T = (Wo@Wv)^T is
      precomputed on host in float64.
    - the A@bk logit shift is constant along the stage-2 softmax axis and
      cancels; it is dropped entirely.
    - stage-1 softmax normalizer r1 applied as a per-row scale on the final
      output tile; stage-2 normalizer r2 applied when evicting (S2@x) from PSUM.

Perf structure (v2):
    - agent products (Asum, AWQ^T, AWK^T, c1) computed ONCE for all 8 local
      batches with the 8*16=128 (batch,agent) pairs as a full 128-wide matmul
      free/partition dim -- 48 full-width matmuls instead of 448 16-wide ones.
    - AWQ^T/AWK^T stored interleaved as one [128, 32] stationary per
      (c-chunk, batch): stage-1 and stage-2 logits come out of ONE x^T stream.
    - per batch ONE pair of XBAR transposes moves [32, 1024] E1/E2 rows into
      token-partition layout (feeds both the ex matmul and the r1 reduce).
    - all HBM<->SBUF transfers are host-permuted so every SBUF partition
      reads/writes a single contiguous 8KB block (large DMA packets).
"""

import sys

if "/opt/trn_rl_repo" not in sys.path:
    sys.path.insert(0, "/opt/trn_rl_repo")

import numpy as np
import ml_dtypes

import concourse.bass as bass
import concourse.mybir as mybir
import concourse.tile as tile
from concourse import bacc
from concourse.bass import ts, ds
from concourse.bass_utils import run_bass_kernel_spmd
from concourse.masks import make_identity

BF16 = mybir.dt.bfloat16
F32 = mybir.dt.float32
F8 = mybir.dt.float8e4

N_CORES = 8
B = 64
B_LOC = B // N_CORES  # 8 batches per core
N = 1024              # tokens
C = 512               # channels
M = 16                # agents
G = N // M            # 64-token pooling groups
P = 128
ND = C // P           # 4 channel chunks
NN = N // P           # 8 token chunks of 128
J = B_LOC * M         # 128 stacked (batch, agent) columns
SCALE = 1.0 / (G * np.sqrt(C))  # logit scale (1/64 pooling fold included)

# test harness may override (e.g. {"trace": True, "tmpdir": ...})
_RUN_KWARGS = {}
_LAST_RESULTS = None


def _build_program():
    nc = bacc.Bacc("TRN2", target_bir_lowering=False, debug=False,
                   num_devices=N_CORES)

    xt_d = nc.dram_tensor("xt", [B_LOC, P, 2, 2, N], F8, kind="ExternalInput")
    xn_d = nc.dram_tensor("xn", [B_LOC, P, NN, C], BF16, kind="ExternalInput")
    xs_d = nc.dram_tensor("xs", [P, ND, J], BF16, kind="ExternalInput")
    wqT_d = nc.dram_tensor("wqT", [P, ND, C], BF16, kind="ExternalInput")
    wqN_d = nc.dram_tensor("wqN", [P, ND, C], BF16, kind="ExternalInput")
    wkN_d = nc.dram_tensor("wkN", [P, ND, C], BF16, kind="ExternalInput")
    wvo_d = nc.dram_tensor("wvoT", [P, ND, C], BF16, kind="ExternalInput")
    bq64_d = nc.dram_tensor("bq64", [P, ND], F32, kind="ExternalInput")
    bp_d = nc.dram_tensor("bp", [P, C], BF16, kind="ExternalInput")
    c12b_d = nc.dram_tensor("c12b", [3 * M, B_LOC], F32, kind="ExternalInput")
    out_d = nc.dram_tensor("out", [B_LOC, P, NN, C], BF16, kind="ExternalOutput")

    with tile.TileContext(nc) as tc:
        with (
            tc.tile_pool(name="const", bufs=1) as const,
            tc.tile_pool(name="pxt", bufs=4) as pxt,
            tc.tile_pool(name="pxn", bufs=4) as pxn,
            tc.tile_pool(name="pe12", bufs=3) as pe12,
            tc.tile_pool(name="pe12p", bufs=3) as pe12p,
            tc.tile_pool(name="psmall", bufs=4) as psmall,
            tc.tile_pool(name="pout", bufs=3) as pout,
            tc.tile_pool(name="ps_big", bufs=3, space="PSUM") as ps_big,
            tc.tile_pool(name="ps_log", bufs=2, space="PSUM") as ps_log,
            tc.tile_pool(name="ps_se", bufs=3, space="PSUM") as ps_se,
        ):
            wqT_s = const.tile([P, ND, C], BF16)
            wqN_s = const.tile([P, ND, C], BF16)
            wkN_s = const.tile([P, ND, C], BF16)
            wvo_s = const.tile([P, ND, C], BF16)
            xs_s = const.tile([P, ND, J], BF16)
            bq64_s = const.tile([P, ND], F32)
            bp_s = const.tile([P, C], BF16)
            asum_s = const.tile([P, ND, J], BF16)
            awqk_s = const.tile([P, 2, 2, B_LOC, 3 * M], F8)
            c12b_s = const.tile([3 * M, B_LOC], F32)
            ident = const.tile([M, M], BF16)

            # first wave of const loads: everything the agent stage needs
            nc.sync.dma_start(wqT_s[:], wqT_d.ap())
            nc.sync.dma_start(xs_s[:], xs_d.ap())
            nc.sync.dma_start(bq64_s[:], bq64_d.ap())
            make_identity(nc, ident)
            nc.vector.memset(awqk_s[:, :, :, :, M:2 * M], 0.0)

            st = [dict() for _ in range(B_LOC)]

            def load_xt(b):
                s = st[b]
                s["xt"] = xt = pxt.tile([P, 2, 2, N], F8, tag="xt", name=f"xt_{b}")
                nc.sync.dma_start(xt[:], xt_d.ap()[b])

            def load_xn(b):
                s = st[b]
                s["xn"] = xn = pxn.tile([P, NN, C], BF16, tag="xn", name=f"xn_{b}")
                nc.sync.dma_start(xn[:], xn_d.ap()[b])

            def load_x(b):
                load_xt(b)
                load_xn(b)

            def agent_stage():
                # AsumT[d, j] = sum_c Wq^T[c, d] xsum^T[c, j] + 64 bq[d]
                for d in range(ND):
                    ps = ps_big.tile([P, B_LOC, M], F32, tag="mm", name="ag")
                    for c in range(ND):
                        nc.tensor.matmul(
                            ps[:], wqT_s[:, c, ds(d * P, P)], xs_s[:, c, :],
                            start=(c == 0), stop=(c == ND - 1))
                    nc.scalar.activation(
                        asum_s[:, d, :], ps[:],
                        mybir.ActivationFunctionType.Identity,
                        bias=bq64_s[:, d:d + 1])
                # AWQ^T[c, j] and AWK^T[c, j], interleaved per batch as the
                # fused [128, 32] logit stationary
                for (w_s, half) in ((wqN_s, 0), (wkN_s, 2)):
                    for c in range(ND):
                        ps = ps_big.tile([P, B_LOC, M], F32, tag="mm", name="ag")
                        for d in range(ND):
                            nc.tensor.matmul(
                                ps[:], w_s[:, d, ds(c * P, P)], asum_s[:, d, :],
                                start=(d == 0), stop=(d == ND - 1))
                        nc.scalar.activation(
                            awqk_s[:, c // 2, c % 2, :,
                                   half * M:(half + 1) * M], ps[:],
                            mybir.ActivationFunctionType.Copy)

            def l12(b):
                # E12^T[0:16, n] = exp(s*Q A^T + c1) ; [16:32, n] = exp(s*A K^T)
                s = st[b]
                s["e12t"] = e12t = pe12.tile([3 * M, N], BF16, tag="e12t",
                                             name=f"e12t_{b}")
                d2 = psmall.tile([3 * M, 2], F32, tag="d2", name=f"d2_{b}")
                for ni in range(2):
                    lg = ps_log.tile([3 * M, 512], F32, tag="log", name=f"log_{b}")
                    for kk in range(2):
                        nc.tensor.matmul(
                            lg[:], awqk_s[:, kk, :, b, :],
                            s["xt"][:, kk, :, ts(ni, 512)],
                            start=(kk == 0), stop=(kk == 1),
                            perf_mode=mybir.MatmulPerfMode.DoubleRow)
                    nc.scalar.activation(
                        e12t[:, ts(ni, 512)], lg[:],
                        mybir.ActivationFunctionType.Exp,
                        bias=c12b_s[:, b:b + 1], scale=float(SCALE),
                        accum_out=d2[:, ni:ni + 1])
                s["d2"] = d2

            def r2c(b):
                s = st[b]
                d2 = s["d2"]
                d2s = psmall.tile([M, 1], F32, tag="d2s", name=f"d2s_{b}")
                nc.vector.tensor_add(d2s[:], d2[2 * M:3 * M, 0:1], d2[2 * M:3 * M, 1:2])
                s["r2"] = r2 = psmall.tile([M, 1], F32, tag="r2", name=f"r2_{b}")
                nc.vector.reciprocal(r2[:], d2s[:])

            def tr(b):
                # e12p[p, o, i] = E12^T[i, o*128+p] via XBAR dma transpose
                s = st[b]
                s["e12p"] = e12p = pe12p.tile([P, NN, 3 * M], BF16, tag="e12p",
                                              name=f"e12p_{b}")
                e12t = s["e12t"]
                h = NN // 2
                nc.sync.dma_start_transpose(e12p[:, 0:h, :], e12t[:, 0:512])
                nc.sync.dma_start_transpose(e12p[:, h:NN, :], e12t[:, 512:])

            def r1(b):
                # r1[n] = sum_m E1^T[m, n]; free-dim reduce in token-partition
                s = st[b]
                r_s = psmall.tile([P, NN], F32, tag="r_s", name=f"r_s_{b}")
                nc.vector.reduce_sum(r_s[:], s["e12p"][:, :, 0:M],
                                     axis=mybir.AxisListType.X)
                s["r_inv"] = r_inv = psmall.tile([P, NN], F32, tag="r_inv",
                                                 name=f"r_inv_{b}")
                nc.vector.reciprocal(r_inv[:], r_s[:])

            def ex(b):
                # exr = diag(r2) * (E2 @ x)   [16, C]
                s = st[b]
                pse = ps_se.tile([M, C], F32, tag="se", name=f"se_{b}")
                for n in range(NN):
                    nc.tensor.matmul(
                        pse[:], s["e12p"][:, n, 2 * M:3 * M], s["xn"][:, n, :],
                        start=(n == 0), stop=(n == NN - 1))
                s["exr"] = exr = psmall.tile([M, C], BF16, tag="exr",
                                             name=f"exr_{b}")
                nc.scalar.activation(
                    exr[:], pse[:], mybir.ActivationFunctionType.Copy,
                    scale=s["r2"][:])

            def exrT(b):
                # exrT[p, c, m] = exr[m, c*128+p] via PE transposes
                s = st[b]
                pst = ps_se.tile([P, ND, M], BF16, tag="se", name=f"tr_{b}")
                for c in range(ND):
                    nc.tensor.transpose(pst[:, c, :], s["exr"][:, ts(c, P)],
                                        ident[:])
                s["exrT"] = exrT_ = psmall.tile([P, ND, M], BF16, tag="exrT",
                                                name=f"exrT_{b}")
                nc.vector.tensor_copy(exrT_[:], pst[:])

            def afw(b):
                # afw = exr @ Wvo^T   [16, C]
                s = st[b]
                psa = ps_se.tile([M, C], F32, tag="se", name=f"afw_{b}")
                for c in range(ND):
                    nc.tensor.matmul(
                        psa[:], s["exrT"][:, c, :], wvo_s[:, c, :],
                        start=(c == 0), stop=(c == ND - 1))
                s["afw"] = afw_ = psmall.tile([M, C], BF16, tag="afw",
                                              name=f"afw_{b}")
                nc.scalar.activation(
                    afw_[:], psa[:], mybir.ActivationFunctionType.Copy)
                nc.vector.tensor_add(afw_[:], afw_[:], bp_s[0:M, :])

            def outp(b, rng=None):
                # out chunk = r1 * (E1 @ afw) + b'
                s = st[b]
                if rng is None:
                    s["o"] = o_s = pout.tile([P, NN, C], BF16, tag="o", name=f"o_{b}")
                    rng = range(NN)
                else:
                    if "o" not in s:
                        s["o"] = pout.tile([P, NN, C], BF16, tag="o", name=f"o_{b}")
                    o_s = s["o"]
                for n in rng:
                    po = ps_big.tile([P, C], F32, tag="mm", name=f"mm_{b}")
                    nc.tensor.matmul(
                        po[:], s["e12t"][0:M, ts(n, P)], s["afw"][:],
                        start=True, stop=True)
                    if n % 2 == 0:
                        nc.scalar.activation(
                            o_s[:, n, :], po[:],
                            mybir.ActivationFunctionType.Copy,
                            scale=s["r_inv"][:, n:n + 1])
                    else:
                        nc.vector.tensor_scalar_mul(
                            o_s[:, n, :], po[:], s["r_inv"][:, n:n + 1])

            def store(b, half=None):
                if half is None:
                    nc.sync.dma_start(out_d.ap()[b], st[b]["o"][:])
                else:
                    h = NN // 2
                    sl = slice(half * h, (half + 1) * h)
                    nc.sync.dma_start(out_d.ap()[b][:, sl], st[b]["o"][:, sl])

            # prologue: agent products overlap the first x loads;
            # xt before xn (xn is consumed one pipeline stage later)
            load_xt(0)
            nc.sync.dma_start(wqN_s[:], wqN_d.ap())
            nc.sync.dma_start(wkN_s[:], wkN_d.ap())
            nc.sync.dma_start(c12b_s[:], c12b_d.ap())
            agent_stage()
            load_xt(1)
            load_xn(0)
            load_xt(2)
            nc.sync.dma_start(wvo_s[:], wvo_d.ap())
            nc.sync.dma_start(bp_s[:], bp_d.ap())
            load_xn(1)
            load_xn(2)

            for b in range(B_LOC):
                l12(b)
                tr(b)
                if b + 3 < B_LOC:
                    load_x(b + 3)
                if b >= 1:
                    ex(b - 1)
                if b >= 2:
                    outp(b - 2)
                    store(b - 2)
                if b >= 1:
                    exrT(b - 1)
                    afw(b - 1)
                r2c(b)
                r1(b)
            L = B_LOC - 1
            ex(L)
            outp(L - 1)
            store(L - 1)
            exrT(L)
            afw(L)
            outp(L, range(0, NN // 2))
            store(L, 0)
            outp(L, range(NN // 2, NN))
            store(L, 1)

    nc.compile()
    return nc


def _prep_inputs(x, Wq, bq, Wk, bk, Wv, bv, Wo, bo):
    bf = ml_dtypes.bfloat16
    x32 = np.asarray(x, np.float32)
    # xt[b, p, kk, i, n] = x[b, n, kk*256+i*128+p]  (fp8 DoubleRow planes)
    f8 = ml_dtypes.float8_e4m3
    xt = np.ascontiguousarray(
        x32.transpose(0, 2, 1).reshape(B, 2, 2, P, N).transpose(0, 3, 1, 2, 4)
    ).astype(f8)
    # xn[b, p, o, c] = x[b, o*128+p, c]
    xn = np.ascontiguousarray(
        x32.reshape(B, NN, P, C).transpose(0, 2, 1, 3)).astype(bf)
    # pooled sums, all local batches stacked: xs[p, o, b*16+m]
    xsum = x32.reshape(B, M, G, C).sum(axis=2)  # [B, M, C]
    Wo64 = np.asarray(Wo, np.float64)
    Wv64 = np.asarray(Wv, np.float64)

    def wtile(w):  # [C, C] -> [P, ND, C] with w[p, o, d] = W[o*128+p, d]
        return np.ascontiguousarray(
            np.asarray(w, np.float32).reshape(ND, P, C).transpose(1, 0, 2)
        ).astype(bf)

    shared = {
        "wqT": wtile(np.asarray(Wq, np.float32).T),
        "wqN": wtile(np.asarray(Wq, np.float32)),
        "wkN": wtile(np.asarray(Wk, np.float32)),
        "wvoT": wtile((Wo64 @ Wv64).T.astype(np.float32)),
        "bq64": np.ascontiguousarray(
            (64.0 * np.asarray(bq, np.float32)).reshape(ND, P).T),
    }
    bprime = np.asarray(bo, np.float64) + Wo64 @ np.asarray(bv, np.float64)
    shared["bp"] = np.tile(bprime.astype(np.float32), (P, 1)).astype(bf)
    in_maps = []
    for ci in range(N_CORES):
        m = dict(shared)
        m["xt"] = np.ascontiguousarray(xt[ci * B_LOC:(ci + 1) * B_LOC])
        m["xn"] = np.ascontiguousarray(xn[ci * B_LOC:(ci + 1) * B_LOC])
        xs_c = xsum[ci * B_LOC:(ci + 1) * B_LOC]  # [B_LOC, M, C]
        xs_j = xs_c.transpose(2, 0, 1).reshape(C, J)  # [C, J], j = b*16+m
        m["xs"] = np.ascontiguousarray(
            xs_j.reshape(ND, P, J).transpose(1, 0, 2)).astype(bf)
        # stage-1 logit bias c1[b, m] = SCALE * (Asum_b,m . bq), exact on host
        asum64 = xs_c.astype(np.float64) @ np.asarray(Wq, np.float64).T \
            + 64.0 * np.asarray(bq, np.float64)
        c1 = SCALE * (asum64 @ np.asarray(bq, np.float64))  # [B_LOC, M]
        c12b = np.zeros((3 * M, B_LOC), np.float32)
        c12b[0:M, :] = c1.T.astype(np.float32)
        m["c12b"] = c12b
        in_maps.append(m)
    return in_maps


def _unpermute_out(res):
    # out_d[b, p, o, c] = out[b, o*128+p, c]
    outs = []
    for ci in range(N_CORES):
        o = np.asarray(res.results[ci]["out"], np.float32)  # [B_LOC, P, NN, C]
        outs.append(o.transpose(0, 2, 1, 3).reshape(B_LOC, N, C))
    return np.concatenate(outs, axis=0)


def kernel(x, Wq, bq, Wk, bk, Wv, bv, Wo, bo):
    global _LAST_RESULTS
    nc = _build_program()
    in_maps = _prep_inputs(x, Wq, bq, Wk, bk, Wv, bv, Wo, bo)
    res = run_bass_kernel_spmd(nc, in_maps, list(range(N_CORES)), **_RUN_KWARGS)
    _LAST_RESULTS = res
    return _unpermute_out(res)
